# revision 27
# baseline (speedup 1.0000x reference)
"""Distance-weighted embedding loss on 8 Trainium2 NeuronCores.

reference:
    gathered = embedding[indices]                      # [B, K, D]
    sq = sum((gathered - emb_batch[:,None,:])**2, -1)  # [B, K]
    loss = sum(sq * attr_sim) / B                      # scalar

Sharding: data-parallel over the batch. Each of the 8 cores handles
B/8 = 512 samples; the embedding table is replicated. Each core reduces
its shard to a single partial sum on-device; the host adds the 8
partials and divides by B (the scalar all-reduce).

Per-core device program (Tile framework):
  - samples processed in 4 groups of 128 (partition dim = sample)
  - per group, the K=50 neighbor columns are gathered 10 at a time via
    indirect DMA: out tile [128, 10*128] f32 where partition p, block j
    holds embedding[indices[g*128+p, t*10+j]]
  - diff = gathered - x (x broadcast along the 10 blocks), square on the
    scalar engine, segmented row-reduce -> sq [128, 10]
  - after 50 columns: loss_g[p] = sum_k attr[p,k]*sq[p,k]
  - final: gpsimd partition-reduce of the [128, 4] per-sample losses
    into a [1, 1] scalar.
"""

import ml_dtypes
import numpy as np

import concourse.bass as bass
import concourse.tile as tile
from concourse import bacc, bass_isa, mybir
from concourse.bass_utils import run_bass_kernel_spmd

F32 = mybir.dt.float32
BF16 = mybir.dt.bfloat16
I32 = mybir.dt.int32

NCORES = 8
D = 128
P = 128
NCOL = 10


def build_program(V: int, S_C: int, K: int, ncol: int):
    """Build the per-core Bass program.

    V: table rows; S_C: samples per core (multiple of 128);
    K: neighbors per sample; ncol: gather columns per indirect DMA.
    """
    G = S_C // P
    assert S_C % P == 0 and K % ncol == 0
    NT = K // ncol

    nc = bacc.Bacc("TRN2", target_bir_lowering=False, debug=False)

    emb_b = nc.dram_tensor("emb_batch", [S_C, D], F32, kind="ExternalInput")
    attr = nc.dram_tensor("attr_sim", [S_C, K], F32, kind="ExternalInput")
    offs = nc.dram_tensor("offsets", [P, G * K], I32, kind="ExternalInput")
    # the table is shipped as bf16 (the pipeline rounds every gathered row
    # to bf16 anyway) — halves the HBM bytes per gathered row
    table = nc.dram_tensor("embedding", [V, D], BF16, kind="ExternalInput")
    loss = nc.dram_tensor("loss", [1, 1], F32, kind="ExternalOutput")

    with tile.TileContext(nc) as tc:
        with (
            tc.tile_pool(name="const", bufs=1) as const,
            tc.tile_pool(name="gather", bufs=8) as gpool,
            tc.tile_pool(name="diff", bufs=3) as dpool,
            tc.tile_pool(name="sq", bufs=3) as spool,
            tc.tile_pool(name="small", bufs=2) as small,
        ):
            # only the first gather's offsets gate the pipeline start: load
            # that column block first, alone on the sync queue
            offs_sb = const.tile([P, G * K], I32)
            nc.sync.dma_start(out=offs_sb[:, :ncol], in_=offs[:, :ncol])
            nc.sync.dma_start(out=offs_sb[:, ncol:], in_=offs[:, ncol:])
            # batch embeddings straight to bf16 (SWDGE cast-during-DMA);
            # the subtract runs in the DVE's packed 2x bf16 mode
            xg_bf = const.tile([P, G * D], BF16)
            nc.gpsimd.dma_start(
                out=xg_bf[:].rearrange("p (g d) -> p g d", g=G),
                in_=emb_b[:].rearrange("(g p) d -> p g d", p=P),
            )
            # attr_all[p, g*K:(g+1)*K] = attr_sim[g*128 + p, :], bf16 too
            attr_all = const.tile([P, G * K], BF16)
            nc.gpsimd.dma_start(
                out=attr_all[:].rearrange("p (g k) -> p g k", g=G),
                in_=attr[:].rearrange("(g p) k -> p g k", p=P),
            )
            losses = const.tile([P, G], F32)

            for g in range(G):
                xg_b = (
                    xg_bf[:, g * D:(g + 1) * D]
                    .unsqueeze(1)
                    .to_broadcast([P, ncol, D])
                )
                sq_g = small.tile([P, K], BF16)

                for t in range(NT):
                    m = gpool.tile([P, ncol * D], BF16)
                    nc.gpsimd.indirect_dma_start(
                        out=m[:],
                        out_offset=None,
                        in_=table[:],
                        in_offset=bass.IndirectOffsetOnAxis(
                            ap=offs_sb[:, g * K + t * ncol: g * K + (t + 1) * ncol],
                            axis=0,
                        ),
                    )
                    dt = dpool.tile([P, ncol * D], BF16)
                    nc.vector.tensor_tensor(
                        out=dt[:].rearrange("p (n d) -> p n d", n=ncol),
                        in0=m[:].rearrange("p (n d) -> p n d", n=ncol),
                        in1=xg_b,
                        op=mybir.AluOpType.subtract,
                    )
                    sq = spool.tile([P, ncol * D], BF16)
                    nc.scalar.square(out=sq[:], in_=dt[:])
                    # 3-stage d-reduction: two packed bf16 tensor_tensor adds
                    # of halves (2x mode), then a short TensorReduce. The
                    # monolithic TensorReduce has no 2x uop and costs ~1.5us.
                    sq3 = sq[:].rearrange("p (n d) -> p n d", n=ncol)
                    h1 = spool.tile([P, ncol * (D // 2)], BF16)
                    h13 = h1[:].rearrange("p (n d) -> p n d", n=ncol)
                    nc.vector.tensor_tensor(
                        out=h13, in0=sq3[:, :, :D // 2], in1=sq3[:, :, D // 2:],
                        op=mybir.AluOpType.add,
                    )
                    h2 = spool.tile([P, ncol * (D // 4)], BF16)
                    h23 = h2[:].rearrange("p (n d) -> p n d", n=ncol)
                    nc.vector.tensor_tensor(
                        out=h23, in0=h13[:, :, :D // 4], in1=h13[:, :, D // 4:],
                        op=mybir.AluOpType.add,
                    )
                    with nc.allow_low_precision("sq row-sums are ~256; bf16 "
                                                "partials average out"):
                        nc.vector.tensor_reduce(
                            out=sq_g[:, t * ncol:(t + 1) * ncol],
                            in_=h23,
                            axis=mybir.AxisListType.X,
                            op=mybir.AluOpType.add,
                        )

                prod = small.tile([P, K], BF16)
                nc.vector.tensor_tensor(
                    out=prod[:], in0=sq_g[:],
                    in1=attr_all[:, g * K:(g + 1) * K],
                    op=mybir.AluOpType.mult,
                )
                nc.vector.tensor_reduce(
                    out=losses[:, g:g + 1], in_=prod[:],
                    axis=mybir.AxisListType.X,
                    op=mybir.AluOpType.add,
                )

            with tc.tile_pool(name="psum", bufs=1, space="PSUM") as psum:
                ones = const.tile([P, 1], F32)
                nc.vector.memset(ones[:], 1.0)
                ps = psum.tile([1, G], F32)
                nc.tensor.matmul(
                    out=ps[:], lhsT=ones[:], rhs=losses[:],
                    start=True, stop=True,
                )
                total = const.tile([1, 1], F32)
                nc.vector.tensor_reduce(
                    out=total[:], in_=ps[:],
                    axis=mybir.AxisListType.X,
                    op=mybir.AluOpType.add,
                )
                nc.sync.dma_start(out=loss[:], in_=total[:])

    nc.compile()
    return nc


def shard_inputs(emb_batch, embedding, attr_sim, indices, ncores=NCORES):
    """Build the per-core input maps (layout prep only)."""
    B, K = attr_sim.shape
    s_c = B // ncores
    g = s_c // P
    emb_batch = np.ascontiguousarray(emb_batch, dtype=np.float32)
    attr_sim = np.ascontiguousarray(attr_sim, dtype=np.float32)
    embedding = np.asarray(embedding, dtype=np.float32).astype(ml_dtypes.bfloat16)
    idx = np.asarray(indices).astype(np.int32)

    in_maps = []
    for c in range(ncores):
        idx_c = idx[c * s_c:(c + 1) * s_c]  # [s_c, K]
        # offsets[p, g*K + k] = idx_c[g*128 + p, k]
        offs = np.ascontiguousarray(
            idx_c.reshape(g, P, K).transpose(1, 0, 2).reshape(P, g * K)
        )
        in_maps.append({
            "emb_batch": emb_batch[c * s_c:(c + 1) * s_c],
            "attr_sim": attr_sim[c * s_c:(c + 1) * s_c],
            "offsets": offs,
            "embedding": embedding,
        })
    return in_maps


_cached = {}


def kernel(emb_batch, embedding, attr_sim, indices, beta):
    emb_batch = np.asarray(emb_batch)
    embedding = np.asarray(embedding)
    attr_sim = np.asarray(attr_sim)
    indices = np.asarray(indices)
    B, K = attr_sim.shape
    V = embedding.shape[0]
    key = (V, B // NCORES, K)
    if key not in _cached:
        _cached[key] = build_program(V, B // NCORES, K, ncol=NCOL)
    nc = _cached[key]
    in_maps = shard_inputs(emb_batch, embedding, attr_sim, indices)
    res = run_bass_kernel_spmd(nc, in_maps, list(range(NCORES)))
    partials = [res.results[c]["loss"][0, 0] for c in range(NCORES)]
    return np.float32(np.sum(np.asarray(partials, dtype=np.float64)) / B)


# revision 29
# speedup vs baseline: 1.0038x; 1.0038x over previous
"""Distance-weighted embedding loss on 8 Trainium2 NeuronCores.

reference:
    gathered = embedding[indices]                      # [B, K, D]
    sq = sum((gathered - emb_batch[:,None,:])**2, -1)  # [B, K]
    loss = sum(sq * attr_sim) / B                      # scalar

Sharding: data-parallel over the batch. Each of the 8 cores handles
B/8 = 512 samples; the embedding table is replicated. Each core reduces
its shard to a single partial sum on-device; the host adds the 8
partials and divides by B (the scalar all-reduce).

Per-core device program (Tile framework):
  - samples processed in 4 groups of 128 (partition dim = sample)
  - per group, the K=50 neighbor columns are gathered 10 at a time via
    indirect DMA: out tile [128, 10*128] f32 where partition p, block j
    holds embedding[indices[g*128+p, t*10+j]]
  - diff = gathered - x (x broadcast along the 10 blocks), square on the
    scalar engine, segmented row-reduce -> sq [128, 10]
  - after 50 columns: loss_g[p] = sum_k attr[p,k]*sq[p,k]
  - final: gpsimd partition-reduce of the [128, 4] per-sample losses
    into a [1, 1] scalar.
"""

import ml_dtypes
import numpy as np

import concourse.bass as bass
import concourse.tile as tile
from concourse import bacc, bass_isa, mybir
from concourse.bass_utils import run_bass_kernel_spmd

F32 = mybir.dt.float32
BF16 = mybir.dt.bfloat16
I32 = mybir.dt.int32

NCORES = 8
D = 128
P = 128
NCOL = 10


def build_program(V: int, S_C: int, K: int, ncol: int):
    """Build the per-core Bass program.

    V: table rows; S_C: samples per core (multiple of 128);
    K: neighbors per sample; ncol: gather columns per indirect DMA.
    """
    G = S_C // P
    assert S_C % P == 0 and K % ncol == 0
    NT = K // ncol

    nc = bacc.Bacc("TRN2", target_bir_lowering=False, debug=False)

    emb_b = nc.dram_tensor("emb_batch", [S_C, D], F32, kind="ExternalInput")
    attr = nc.dram_tensor("attr_sim", [S_C, K], F32, kind="ExternalInput")
    offs = nc.dram_tensor("offsets", [P, G * K], I32, kind="ExternalInput")
    # the table is shipped as bf16 (the pipeline rounds every gathered row
    # to bf16 anyway) — halves the HBM bytes per gathered row
    table = nc.dram_tensor("embedding", [V, D], BF16, kind="ExternalInput")
    loss = nc.dram_tensor("loss", [1, 1], F32, kind="ExternalOutput")

    with tile.TileContext(nc) as tc:
        with (
            tc.tile_pool(name="const", bufs=1) as const,
            tc.tile_pool(name="gather", bufs=8) as gpool,
            tc.tile_pool(name="diff", bufs=3) as dpool,
            tc.tile_pool(name="sq", bufs=3) as spool,
            tc.tile_pool(name="small", bufs=2) as small,
        ):
            # only the first gather's offsets gate the pipeline start: load
            # that column block first, alone on the sync queue
            offs_sb = const.tile([P, G * K], I32)
            nc.sync.dma_start(out=offs_sb[:, :ncol], in_=offs[:, :ncol])
            nc.sync.dma_start(out=offs_sb[:, ncol:], in_=offs[:, ncol:])
            # keep Q7 free for gather descriptor-gen: stage the f32 loads on
            # the scalar HWDGE queue, cast to bf16 on the DVE (off critical
            # path). bf16 operands put the subtract in the packed 2x mode.
            xg_all = const.tile([P, G * D], F32)
            nc.scalar.dma_start(
                out=xg_all[:].rearrange("p (g d) -> p g d", g=G),
                in_=emb_b[:].rearrange("(g p) d -> p g d", p=P),
            )
            attr_f = const.tile([P, G * K], F32)
            nc.scalar.dma_start(
                out=attr_f[:].rearrange("p (g k) -> p g k", g=G),
                in_=attr[:].rearrange("(g p) k -> p g k", p=P),
            )
            xg_bf = const.tile([P, G * D], BF16)
            nc.vector.tensor_copy(out=xg_bf[:], in_=xg_all[:])
            attr_all = const.tile([P, G * K], BF16)
            nc.vector.tensor_copy(out=attr_all[:], in_=attr_f[:])
            losses = const.tile([P, G], F32)

            # groups 0..G-2 use wide tiles (fewer instructions -> less
            # per-op + semaphore overhead); the last group uses narrow
            # tiles so the trailing compute chain after the final gather
            # is short.
            if K % 25 == 0:
                wide = [(k, 25) for k in range(0, K, 25)]
            else:
                wide = [(k, ncol) for k in range(0, K, ncol)]
            narrow = [(k, ncol) for k in range(0, K, ncol)]

            for g in range(G):
                segs = narrow if g == G - 1 else wide
                sq_g = small.tile([P, K], BF16)

                for k0, nc_t in segs:
                    xg_b = (
                        xg_bf[:, g * D:(g + 1) * D]
                        .unsqueeze(1)
                        .to_broadcast([P, nc_t, D])
                    )
                    m = gpool.tile([P, 25 * D], BF16, tag="m")
                    mm = m[:, :nc_t * D]
                    nc.gpsimd.indirect_dma_start(
                        out=mm,
                        out_offset=None,
                        in_=table[:],
                        in_offset=bass.IndirectOffsetOnAxis(
                            ap=offs_sb[:, g * K + k0: g * K + k0 + nc_t],
                            axis=0,
                        ),
                    )
                    dt = dpool.tile([P, 25 * D], BF16, tag="dt")
                    dtt = dt[:, :nc_t * D]
                    nc.vector.tensor_tensor(
                        out=dtt.rearrange("p (n d) -> p n d", n=nc_t),
                        in0=mm.rearrange("p (n d) -> p n d", n=nc_t),
                        in1=xg_b,
                        op=mybir.AluOpType.subtract,
                    )
                    sq = spool.tile([P, 25 * D], BF16, tag="sq")
                    sqq = sq[:, :nc_t * D]
                    nc.scalar.square(out=sqq, in_=dtt)
                    # 3-stage d-reduction: two packed bf16 tensor_tensor
                    # adds of halves (2x mode), then a short TensorReduce
                    # (the monolithic TensorReduce has no 2x uop).
                    sq3 = sqq.rearrange("p (n d) -> p n d", n=nc_t)
                    h1 = spool.tile([P, 25 * (D // 2)], BF16, tag="h1")
                    h13 = h1[:, :nc_t * (D // 2)].rearrange(
                        "p (n d) -> p n d", n=nc_t)
                    nc.vector.tensor_tensor(
                        out=h13, in0=sq3[:, :, :D // 2], in1=sq3[:, :, D // 2:],
                        op=mybir.AluOpType.add,
                    )
                    h2 = spool.tile([P, 25 * (D // 4)], BF16, tag="h2")
                    h23 = h2[:, :nc_t * (D // 4)].rearrange(
                        "p (n d) -> p n d", n=nc_t)
                    nc.vector.tensor_tensor(
                        out=h23, in0=h13[:, :, :D // 4], in1=h13[:, :, D // 4:],
                        op=mybir.AluOpType.add,
                    )
                    with nc.allow_low_precision("sq row-sums are ~256; bf16 "
                                                "partials average out"):
                        nc.vector.tensor_reduce(
                            out=sq_g[:, k0:k0 + nc_t],
                            in_=h23,
                            axis=mybir.AxisListType.X,
                            op=mybir.AluOpType.add,
                        )

                prod = small.tile([P, K], BF16)
                nc.vector.tensor_tensor(
                    out=prod[:], in0=sq_g[:],
                    in1=attr_all[:, g * K:(g + 1) * K],
                    op=mybir.AluOpType.mult,
                )
                nc.vector.tensor_reduce(
                    out=losses[:, g:g + 1], in_=prod[:],
                    axis=mybir.AxisListType.X,
                    op=mybir.AluOpType.add,
                )

            with tc.tile_pool(name="psum", bufs=1, space="PSUM") as psum:
                ones = const.tile([P, 1], F32)
                nc.vector.memset(ones[:], 1.0)
                ps = psum.tile([1, G], F32)
                nc.tensor.matmul(
                    out=ps[:], lhsT=ones[:], rhs=losses[:],
                    start=True, stop=True,
                )
                total = const.tile([1, 1], F32)
                nc.vector.tensor_reduce(
                    out=total[:], in_=ps[:],
                    axis=mybir.AxisListType.X,
                    op=mybir.AluOpType.add,
                )
                nc.sync.dma_start(out=loss[:], in_=total[:])

    nc.compile()
    return nc


def shard_inputs(emb_batch, embedding, attr_sim, indices, ncores=NCORES):
    """Build the per-core input maps (layout prep only)."""
    B, K = attr_sim.shape
    s_c = B // ncores
    g = s_c // P
    emb_batch = np.ascontiguousarray(emb_batch, dtype=np.float32)
    attr_sim = np.ascontiguousarray(attr_sim, dtype=np.float32)
    embedding = np.asarray(embedding, dtype=np.float32).astype(ml_dtypes.bfloat16)
    idx = np.asarray(indices).astype(np.int32)

    in_maps = []
    for c in range(ncores):
        idx_c = idx[c * s_c:(c + 1) * s_c]  # [s_c, K]
        # offsets[p, g*K + k] = idx_c[g*128 + p, k]
        offs = np.ascontiguousarray(
            idx_c.reshape(g, P, K).transpose(1, 0, 2).reshape(P, g * K)
        )
        in_maps.append({
            "emb_batch": emb_batch[c * s_c:(c + 1) * s_c],
            "attr_sim": attr_sim[c * s_c:(c + 1) * s_c],
            "offsets": offs,
            "embedding": embedding,
        })
    return in_maps


_cached = {}


def kernel(emb_batch, embedding, attr_sim, indices, beta):
    emb_batch = np.asarray(emb_batch)
    embedding = np.asarray(embedding)
    attr_sim = np.asarray(attr_sim)
    indices = np.asarray(indices)
    B, K = attr_sim.shape
    V = embedding.shape[0]
    key = (V, B // NCORES, K)
    if key not in _cached:
        _cached[key] = build_program(V, B // NCORES, K, ncol=NCOL)
    nc = _cached[key]
    in_maps = shard_inputs(emb_batch, embedding, attr_sim, indices)
    res = run_bass_kernel_spmd(nc, in_maps, list(range(NCORES)))
    partials = [res.results[c]["loss"][0, 0] for c in range(NCORES)]
    return np.float32(np.sum(np.asarray(partials, dtype=np.float64)) / B)
